# revision 2
# baseline (speedup 1.0000x reference)
"""Bilinear RoI pooling (grid_sample style) on 8 Trainium2 NeuronCores.

Strategy (data-parallel over boxes, per sharding hint):
  - feats [512, 64, 256] f32 is transposed host-side to [H*W, 512] so one
    sample point's channel vector is contiguous (2KB), and replicated to all
    8 cores. boxes [2048, 4] is sharded 256/core.
  - On device, per core: box -> affine params (DVE), broadcast to the 12544
    flat sample points via an SWDGE dma_gather from a small DRAM params
    table, then sample coords / bilinear weights / gather indices are
    computed with DVE ops in a flat [128, 98] layout.
  - The 4 bilinear corners are fetched with one big SWDGE dma_gather stream
    (50176 descriptors x 2KB) in an order that lands corner q of point m of
    each 32-point group in K-partition q*32+m.
  - A PE matmul per (32-point group, 128-channel chunk) with a sparse
    [128, 32] weight matrix (delta(k%32==n) * w_q(pt)) does the whole
    weighted 4-corner reduction, producing [channels, points] tiles directly
    in the output layout. PSUM -> SBUF -> DRAM with 196B-contiguous runs.
"""
import sys
import numpy as np

sys.path.insert(0, "/opt/trn_rl_repo")

OH = OW = 7
C, H, W = 512, 64, 256
HW = H * W
CC = C // 128
B_TOTAL = 2048
N_CORES = 8
B_LOCAL = B_TOTAL // N_CORES


def _host_constants(Blocal):
    NPTS = Blocal * OH * OW
    assert NPTS % 128 == 0
    G = NPTS // 128
    NG32 = NPTS // 32
    NIDX = NPTS * 4
    WCOLS = NIDX // 16
    pts = np.arange(NPTS)
    p = pts % 49
    lin = np.linspace(-1.0, 1.0, 7).astype(np.float32)
    gxf = lin[p % 7].reshape(G, 128).T.astype(np.float32).copy()
    gyf = lin[p // 7].reshape(G, 128).T.astype(np.float32).copy()
    bidx = (pts // 49).astype(np.int16)
    pidxw = np.zeros((16, NPTS // 16), np.int16)
    pidxw[pts % 16, pts // 16] = bidx
    mask2 = np.zeros((128, 32), np.float32)
    for k in range(128):
        mask2[k, k % 32] = 1.0
    return dict(gxf=gxf, gyf=gyf, pidxw=pidxw, mask2=mask2,
                NPTS=NPTS, G=G, NG32=NG32, NIDX=NIDX, WCOLS=WCOLS)


def _build(nc, tc, Blocal, Him, Wim, fdt, chunk_g32=8, seg_g32=16,
           stage_pts=1024):
    from contextlib import ExitStack
    import concourse.mybir as mybir
    from concourse import bass

    cst = _host_constants(Blocal)
    NPTS, G, NG32, NIDX, WCOLS = (cst[k] for k in
                                  ("NPTS", "G", "NG32", "NIDX", "WCOLS"))
    f32 = mybir.dt.float32

    feats_t = nc.dram_tensor("feats_t", [HW, C], fdt, kind="ExternalInput")
    boxes = nc.dram_tensor("boxes", [Blocal, 4], f32, kind="ExternalInput")
    gxf_d = nc.dram_tensor("gxf", [128, G], f32, kind="ExternalInput")
    gyf_d = nc.dram_tensor("gyf", [128, G], f32, kind="ExternalInput")
    pidxw_d = nc.dram_tensor("pidxw", [16, NPTS // 16], mybir.dt.int16,
                             kind="ExternalInput")
    mask2_d = nc.dram_tensor("mask2", [128, 32], f32, kind="ExternalInput")
    out_d = nc.dram_tensor("out", [Blocal, C, 49], f32, kind="ExternalOutput")
    params64 = nc.dram_tensor("params64", [Blocal, 64], f32)
    wdram = nc.dram_tensor("wdram", [16, WCOLS], mybir.dt.int16)

    cax = np.float32(0.5 * (W - 1) / (Wim - 1))
    cay = np.float32(0.5 * (H - 1) / (Him - 1))

    es = ExitStack()
    raw = lambda name, shape, dt: es.enter_context(nc.sbuf_tensor(name, shape, dt))
    A = mybir.AluOpType

    BH = Blocal // 128
    btile = raw("btile", [128, BH, 4], f32)
    P64 = raw("P64", [128, BH, 64], f32)
    gxf_s = raw("gxf_s", [128, G], f32)
    gyf_s = raw("gyf_s", [128, G], f32)
    pidx_s = raw("pidx_s", [128, NPTS // 16], mybir.dt.int16)
    mask_s = raw("mask_s", [128, 32], f32)
    pflat = raw("pflat", [128, G, 64], f32)
    wrapped = raw("wrapped", [128, WCOLS], mybir.dt.int16)
    Wi = raw("Wi", [128, NG32], f32)
    cnames = ["ix", "x0f", "wx", "x1f", "iy", "y0f", "wy", "y1f",
              "ux", "uy", "t0", "gtt", "yb0", "yb1"]
    ct = {n: raw("c_" + n, [128, G], f32) for n in cnames}
    x0i = raw("c_x0i", [128, G], mybir.dt.int32)
    wq = [raw(f"c_w{q}", [128, G], f32) for q in range(4)]
    idxq = [raw(f"c_i{q}", [128, G], mybir.dt.int16) for q in range(4)]
    pp = G * 64

    with tc.tile_pool(name="sbuf", bufs=2) as pool, \
         tc.tile_pool(name="gpool", bufs=3) as gpool, \
         tc.tile_pool(name="spool", bufs=2) as spool, \
         tc.tile_pool(name="psum", bufs=2, space="PSUM") as psum_pool:
        nc.sync.dma_start(out=gxf_s[:, :], in_=gxf_d[:, :])
        nc.sync.dma_start(out=gyf_s[:, :], in_=gyf_d[:, :])
        nc.sync.dma_start(out=mask_s[:, :], in_=mask2_d[:, :])
        for rep in range(8):
            nc.sync.dma_start(
                out=bass.AP(pidx_s, rep * 16 * (NPTS // 16),
                            [[NPTS // 16, 16], [1, NPTS // 16]]),
                in_=pidxw_d[:, :])
        nc.sync.dma_start(
            out=btile[:, :, :],
            in_=bass.AP(boxes, 0, [[4, 128], [128 * 4, BH], [1, 4]]))

        nc.vector.memset(P64[:, :, :], 0.0)
        nc.vector.tensor_scalar(out=P64[:, :, 0:1], in0=btile[:, :, 2:3],
                                scalar1=1.0, scalar2=float(cax),
                                op0=A.subtract, op1=A.mult)
        nc.vector.tensor_scalar(out=P64[:, :, 1:2], in0=btile[:, :, 0:1],
                                scalar1=float(2 * cax), scalar2=float(2 * cax),
                                op0=A.mult, op1=A.subtract)
        nc.vector.tensor_scalar(out=P64[:, :, 2:3], in0=btile[:, :, 3:4],
                                scalar1=1.0, scalar2=float(cay),
                                op0=A.subtract, op1=A.mult)
        nc.vector.tensor_scalar(out=P64[:, :, 3:4], in0=btile[:, :, 1:2],
                                scalar1=float(2 * cay), scalar2=float(2 * cay),
                                op0=A.mult, op1=A.subtract)
        nc.sync.dma_start(
            out=bass.AP(params64, 0, [[64, 128], [128 * 64, BH], [1, 64]]),
            in_=P64[:, :, :])
        PCH = 1024
        for c0 in range(0, NPTS, PCH):
            n = min(PCH, NPTS - c0)
            nc.gpsimd.dma_gather(
                out_ap=pflat[:, c0 // 128:(c0 + n) // 128, :],
                in_ap=params64[:, :],
                idxs_ap=pidx_s[:, c0 // 16:(c0 + n) // 16],
                num_idxs=n, num_idxs_reg=n, elem_size=64)

        Ax = bass.AP(pflat, 0, [[pp, 128], [64, G]])
        Bx = bass.AP(pflat, 1, [[pp, 128], [64, G]])
        Ay = bass.AP(pflat, 2, [[pp, 128], [64, G]])
        By = bass.AP(pflat, 3, [[pp, 128], [64, G]])

        V = nc.vector

        def coord(gA, pA, pB, hi, o_if, o_f0, o_w, o_f1):
            V.tensor_tensor(out=ct["t0"][:, :], in0=gA[:, :], in1=pA, op=A.mult)
            V.tensor_tensor(out=ct[o_if][:, :], in0=ct["t0"][:, :], in1=pB,
                            op=A.add)
            V.tensor_scalar(out=ct[o_if][:, :], in0=ct[o_if][:, :],
                            scalar1=0.0, scalar2=float(hi), op0=A.max,
                            op1=A.min)
            V.tensor_copy(out=x0i[:, :], in_=ct[o_if][:, :])
            V.tensor_copy(out=ct[o_f0][:, :], in_=x0i[:, :])
            V.tensor_tensor(out=ct["gtt"][:, :], in0=ct[o_f0][:, :],
                            in1=ct[o_if][:, :], op=A.is_gt)
            V.tensor_tensor(out=ct[o_f0][:, :], in0=ct[o_f0][:, :],
                            in1=ct["gtt"][:, :], op=A.subtract)
            V.tensor_tensor(out=ct[o_w][:, :], in0=ct[o_if][:, :],
                            in1=ct[o_f0][:, :], op=A.subtract)
            V.tensor_scalar(out=ct[o_f1][:, :], in0=ct[o_f0][:, :],
                            scalar1=1.0, scalar2=float(hi), op0=A.add,
                            op1=A.min)

        coord(gxf_s, Ax, Bx, W - 1, "ix", "x0f", "wx", "x1f")
        coord(gyf_s, Ay, By, H - 1, "iy", "y0f", "wy", "y1f")
        V.tensor_scalar(out=ct["ux"][:, :], in0=ct["wx"][:, :],
                        scalar1=-1.0, scalar2=1.0, op0=A.mult, op1=A.add)
        V.tensor_scalar(out=ct["uy"][:, :], in0=ct["wy"][:, :],
                        scalar1=-1.0, scalar2=1.0, op0=A.mult, op1=A.add)
        V.tensor_tensor(out=wq[0][:, :], in0=ct["ux"][:, :],
                        in1=ct["uy"][:, :], op=A.mult)
        V.tensor_tensor(out=wq[1][:, :], in0=ct["wx"][:, :],
                        in1=ct["uy"][:, :], op=A.mult)
        V.tensor_tensor(out=wq[2][:, :], in0=ct["ux"][:, :],
                        in1=ct["wy"][:, :], op=A.mult)
        V.tensor_tensor(out=wq[3][:, :], in0=ct["wx"][:, :],
                        in1=ct["wy"][:, :], op=A.mult)
        V.tensor_scalar(out=ct["yb0"][:, :], in0=ct["y0f"][:, :],
                        scalar1=float(W), scalar2=None, op0=A.mult)
        V.tensor_scalar(out=ct["yb1"][:, :], in0=ct["y1f"][:, :],
                        scalar1=float(W), scalar2=None, op0=A.mult)
        for q, (ya, xa) in enumerate([("yb0", "x0f"), ("yb0", "x1f"),
                                      ("yb1", "x0f"), ("yb1", "x1f")]):
            V.tensor_tensor(out=ct["t0"][:, :], in0=ct[ya][:, :],
                            in1=ct[xa][:, :], op=A.add)
            V.tensor_copy(out=idxq[q][:, :], in_=ct["t0"][:, :])

        with nc.allow_non_contiguous_dma(reason="wrapped/Wi build"):
            for q in range(4):
                for u2 in range(4):
                    for h5 in range(2):
                        src = bass.AP(idxq[q], (u2 * 32 + h5 * 16) * G,
                                      [[G, 16], [1, G]])
                        dst = bass.AP(wdram, 8 * u2 + 2 * q + h5,
                                      [[WCOLS, 16], [32, G]])
                        nc.sync.dma_start(out=dst, in_=src)
            for q in range(4):
                for u2 in range(4):
                    src = bass.AP(wq[q], (32 * u2) * G, [[G, 32], [1, G]])
                    dst = bass.AP(Wi, (q * 32) * NG32 + u2,
                                  [[NG32, 32], [4, G]])
                    nc.sync.dma_start(out=dst, in_=src)
        for rep in range(8):
            nc.sync.dma_start(
                out=bass.AP(wrapped, rep * 16 * WCOLS,
                            [[WCOLS, 16], [1, WCOLS]]),
                in_=bass.AP(wdram, 0, [[WCOLS, 16], [1, WCOLS]]))

        n_seg = (NG32 + seg_g32 - 1) // seg_g32
        seg_pts = seg_g32 * 32
        assert stage_pts % seg_pts == 0
        segs_per_stage = stage_pts // seg_pts
        stage = None
        stage_base = 0

        def flush_stage(stage, base_pt, n_pts):
            sp = stage[:].ap[0][0]
            st = stage[:].tensor
            for cc in range(CC):
                pt0 = base_pt
                end = base_pt + n_pts
                while pt0 < end:
                    b = pt0 // 49
                    p0 = pt0 % 49
                    if p0 != 0 or end - pt0 < 49:
                        npts = min(49 - p0, end - pt0)
                        dst = bass.AP(out_d, b * C * 49 + cc * 128 * 49 + p0,
                                      [[49, 128], [1, npts]])
                        src = bass.AP(st, cc * stage_pts + (pt0 - base_pt),
                                      [[sp, 128], [1, npts]])
                        nc.sync.dma_start(out=dst, in_=src)
                        pt0 += npts
                    else:
                        nb = (end - pt0) // 49
                        dst = bass.AP(out_d, b * C * 49 + cc * 128 * 49,
                                      [[49, 128], [C * 49, nb], [1, 49]])
                        src = bass.AP(st, cc * stage_pts + (pt0 - base_pt),
                                      [[sp, 128], [49, nb], [1, 49]])
                        nc.sync.dma_start(out=dst, in_=src)
                        pt0 += nb * 49

        for seg in range(n_seg):
            g0 = seg * seg_g32
            g1 = min(g0 + seg_g32, NG32)
            if seg % segs_per_stage == 0:
                stage = spool.tile([128, CC, stage_pts], f32, name="stage")
                stage_base = g0 * 32
            psums = [psum_pool.tile([128, 512], f32, name=f"ps{cc}")
                     for cc in range(CC)]
            for ch0 in range(g0, g1, chunk_g32):
                ch1 = min(ch0 + chunk_g32, g1)
                ng = ch1 - ch0
                nidx = ng * 128
                Gt = gpool.tile([128, chunk_g32, C], fdt, name="Gt")
                nc.gpsimd.dma_gather(
                    out_ap=Gt[:, :ng, :], in_ap=feats_t[:, :],
                    idxs_ap=wrapped[:, ch0 * 8: ch0 * 8 + nidx // 16],
                    num_idxs=nidx, num_idxs_reg=nidx, elem_size=C)
                rhs = pool.tile([128, chunk_g32, 32], f32, name="rhs")
                mask_b = bass.AP(mask_s, 0, [[32, 128], [0, ng], [1, 32]])
                wi_b = bass.AP(Wi, ch0, [[NG32, 128], [1, ng], [0, 32]])
                nc.vector.tensor_tensor(out=rhs[:, :ng, :], in0=mask_b,
                                        in1=wi_b, op=A.mult)
                for gi in range(ng):
                    g32 = ch0 + gi
                    col = (g32 - g0) * 32
                    for cc in range(CC):
                        nc.tensor.matmul(
                            out=psums[cc][:, col:col + 32],
                            lhsT=Gt[:, gi, cc * 128:(cc + 1) * 128],
                            rhs=rhs[:, gi, :],
                            start=True, stop=True)
            npts_seg = (g1 - g0) * 32
            soff = g0 * 32 - stage_base
            import concourse.mybir as _mb
            for cc in range(CC):
                dst = stage[:, cc, soff:soff + npts_seg]
                if cc % 2 == 0:
                    nc.vector.tensor_copy(out=dst, in_=psums[cc][:, :npts_seg])
                else:
                    nc.scalar.activation(
                        out=dst, in_=psums[cc][:, :npts_seg],
                        func=_mb.ActivationFunctionType.Copy)
            if (seg + 1) % segs_per_stage == 0 or seg == n_seg - 1:
                flush_stage(stage, stage_base, g1 * 32 - stage_base)
    return cst


_CACHE = {}


def _get_compiled(Him, Wim):
    key = (Him, Wim)
    if key in _CACHE:
        return _CACHE[key]
    import concourse.bacc as bacc
    import concourse.tile as tile
    import concourse.mybir as mybir
    nc = bacc.Bacc("TRN2", target_bir_lowering=False, debug=False)
    with tile.TileContext(nc) as tc:
        cst = _build(nc, tc, B_LOCAL, Him, Wim, mybir.dt.float32)
    nc.compile()
    _CACHE[key] = (nc, cst)
    return nc, cst


def _run(feats, boxes, Him, Wim, trace=False, tmpdir=None):
    from concourse.bass_utils import run_bass_kernel_spmd
    nc, cst = _get_compiled(Him, Wim)
    feats_t = np.ascontiguousarray(
        feats.transpose(1, 2, 0).reshape(HW, C)).astype(np.float32)
    base = {"feats_t": feats_t, "gxf": cst["gxf"], "gyf": cst["gyf"],
            "pidxw": cst["pidxw"], "mask2": cst["mask2"]}
    in_maps = []
    for i in range(N_CORES):
        m = dict(base)
        m["boxes"] = np.ascontiguousarray(
            boxes[i * B_LOCAL:(i + 1) * B_LOCAL]).astype(np.float32)
        in_maps.append(m)
    res = run_bass_kernel_spmd(nc, in_maps, list(range(N_CORES)),
                               trace=trace, tmpdir=tmpdir)
    out = np.concatenate([res.results[i]["out"] for i in range(N_CORES)], 0)
    return out.reshape(B_TOTAL, C, OH, OW), res


def kernel(**inputs):
    feats = np.asarray(inputs["feats"], dtype=np.float32)
    boxes = np.asarray(inputs["boxes"], dtype=np.float32)
    Him = int(inputs["image_height"])
    Wim = int(inputs["image_width"])
    out, _ = _run(feats, boxes, Him, Wim, trace=False)
    return out
